# Initial kernel scaffold
#
"""Causal self-attention block on 8 Trainium2 NeuronCores.

Reference computation (B=4, T=2048, D=1024, H=16, hd=64):
    qkv = x @ Wqkv + bqkv ; per-head causal softmax(q k^T / sqrt(hd)) v ;
    out = concat_heads @ Wproj + bproj

Sharding: core c = (batch b = c//2, head-group g = c%2 of 8 heads).
Each core computes its batch's qkv for its 8 heads, the causal attention,
and a partial projection (its 512 rows of Wproj). Host sums the two
head-group partials per batch and adds bproj.

All matmuls run bf16 x bf16 with fp32 PSUM accumulation (bf16 stationaries
get the compiler's fast-weight-load path; mixing 16/32-bit operands is not
supported by the hardware).

The three phases are fused into one software pipeline per 512-wide t-chunk
so the PE never idles while the scalar engine works through exp():
    ph1(0) | attn(0)+ph1(1) | attn(1)+ph1(2)+proj(0) | ... | proj(3)
Inside attn(j), scores for kj-pair u+1 are emitted before the AV matmuls
of pair u, the two heads of a pair are interleaved, exp() runs on
1024-wide tiles (two kj tiles at once), and QKV/proj "filler" matmuls are
spliced between AV steps at a proportional rate. Softmax denominators come
free from a ones-column in the AV stationary; normalization goes
denominator row -> DMA to DRAM -> DMA partition-broadcast back ->
reciprocal_approx_fast -> multiply (no scalar-engine or PE involvement).
"""

import numpy as np

B, T, D, H, HD = 4, 2048, 1024, 16, 64
NCORES = 8
HPG = H // 2          # heads per group: 8
C = HPG * HD          # per-core q/k/v columns: 512
KO = D // 128         # 8 input-dim k-tiles
NCH = T // 512        # 4 t-chunks
NT = T // 128         # 16 t-tiles
CG = C // 128         # 4 col-groups per q/k
SCALE = 1.0 / np.sqrt(HD)

_CACHE = {}


def _build():
    import functools
    import concourse.mybir as mybir
    import concourse.tile as tile
    from concourse import bacc
    import concourse.bass as bass

    F32 = mybir.dt.float32
    BF16 = mybir.dt.bfloat16
    AF = mybir.ActivationFunctionType

    nc = bacc.Bacc("TRN2", target_bir_lowering=False, debug=False,
                   num_devices=NCORES)
    XBF = nc.declare_dram_parameter("XBF", [D, T], BF16, isOutput=False)
    WQ = nc.declare_dram_parameter("WQ", [D, C], BF16, isOutput=False)
    WK = nc.declare_dram_parameter("WK", [D, C], BF16, isOutput=False)
    WV = nc.declare_dram_parameter("WV", [D, C], BF16, isOutput=False)
    BQ = nc.declare_dram_parameter("BQ", [C], F32, isOutput=False)
    BK = nc.declare_dram_parameter("BK", [C], F32, isOutput=False)
    BV = nc.declare_dram_parameter("BV", [C], F32, isOutput=False)
    WP = nc.declare_dram_parameter("WP", [C, D], BF16, isOutput=False)
    MASKS = nc.declare_dram_parameter("MASKS", [128, 4, 512], BF16,
                                      isOutput=False)
    OUT = nc.declare_dram_parameter("OUT", [T, D], F32, isOutput=True)

    xbf_r = XBF[:, :].rearrange("(ko p) t -> p ko t", p=128)
    wq_r = WQ[:, :].rearrange("(ko p) c -> p ko c", p=128)
    wk_r = WK[:, :].rearrange("(ko p) c -> p ko c", p=128)
    wv_r = WV[:, :].rearrange("(ko p) c -> p ko c", p=128)
    wp_r = WP[:, :].rearrange("(ko p) c -> p ko c", p=128)
    bq_r = BQ[:].rearrange("(cg p) -> p cg", p=128)
    bk_r = BK[:].rearrange("(cg p) -> p cg", p=128)
    bv_ap = BV[:]
    bv_bcast = bass.AP(tensor=bv_ap.tensor, offset=bv_ap.offset,
                       ap=[[0, 128]] + list(bv_ap.ap))

    from contextlib import ExitStack
    with tile.TileContext(nc) as tc:
        with ExitStack() as ctx:
            def pool(name, bufs, space="SBUF"):
                return ctx.enter_context(
                    tc.tile_pool(name=name, bufs=bufs, space=space))
            consts = pool("consts", 1)
            ktp = pool("ktp", 1)
            vtp = pool("vtp", 1)
            ytp = pool("ytp", 1)
            wp1 = pool("wp1", 1)
            xs = pool("xs", 2)
            stg = pool("stg", 3)
            qcp = pool("qc", 2)
            pep = pool("pe", 4)
            rbp = pool("rb", 2)
            rcp = pool("rc", 2)
            ostp = pool("ost", 2)
            dnp = pool("dn", 3)
            dramp = pool("dramp", 1, space="DRAM")
            dramd = pool("dramd", 4, space="DRAM")
            ps1 = pool("ps1", 2, space="PSUM")
            ps_p = pool("psp", 2, space="PSUM")
            ps_oa = pool("psa", 2, space="PSUM")

            WQ_sb = wp1.tile([128, KO, C], BF16)
            WK_sb = wp1.tile([128, KO, C], BF16)
            WV_sb = wp1.tile([128, KO, C], BF16)
            WP_sb = wp1.tile([128, CG, D], BF16)

            masks_sb = consts.tile([128, 4, 512], BF16)
            bq_sb = consts.tile([128, CG], F32)
            bk_sb = consts.tile([128, CG], F32)
            bv_sb = consts.tile([128, C], F32)
            nc.sync.dma_start(out=bq_sb[:], in_=bq_r)
            nc.sync.dma_start(out=bk_sb[:], in_=bk_r)
            nc.sync.dma_start(out=bv_sb[:], in_=bv_bcast)
            nc.sync.dma_start(out=WV_sb[:], in_=wv_r)
            nc.sync.dma_start(out=masks_sb[:], in_=MASKS[:, :, :])
            nc.sync.dma_start(out=WP_sb[:], in_=wp_r)

            kT_sb = ktp.tile([128, CG, T], BF16)      # [kcol%128, cg, t]
            v_aug = vtp.tile([128, NT, HPG, 128], BF16)
            yT_sb = ytp.tile([128, CG, T], BF16)      # [ycol%128, cg, t]
            qt_scr = dramp.tile([C, T], BF16)
            nc.vector.memset(v_aug[:, :, :, HD:128], 1.0)

            # ---- phase-1 step closures for one t-chunk (run as fillers) ---
            def ph1_steps(tci, box=None):
                tsl = slice(512 * tci, 512 * (tci + 1))
                if box is None:
                    box = {}

                def load_x():
                    if "xc" in box:
                        return
                    box["xc"] = xs.tile([128, KO, 512], BF16, tag="xc",
                                        name="xc")
                    nc.gpsimd.dma_start(out=box["xc"][:],
                                        in_=xbf_r[:, :, tsl])

                steps = [load_x]

                def qk_step(which, pair, ko, st_box):
                    W_sb = WQ_sb if which == "q" else WK_sb
                    b_sb = bq_sb if which == "q" else bk_sb
                    if ko == 0:
                        st_box["ps"] = [
                            ps1.tile([128, 512], F32, tag="ph1",
                                     name=f"p{which}{cg}")
                            for cg in pair]
                    for idx, cg in enumerate(pair):
                        nc.tensor.matmul(
                            st_box["ps"][idx][:],
                            W_sb[:, ko, 128 * cg:128 * (cg + 1)],
                            box["xc"][:, ko, :],
                            start=(ko == 0), stop=(ko == KO - 1))
                    if ko == KO - 1:
                        for idx, cg in enumerate(pair):
                            if which == "q":
                                st = stg.tile([128, 512], BF16, tag="stg")
                                nc.vector.tensor_scalar_add(
                                    out=st[:], in0=st_box["ps"][idx][:],
                                    scalar1=b_sb[:, cg:cg + 1])
                                nc.sync.dma_start(
                                    out=qt_scr[128 * cg:128 * (cg + 1), tsl],
                                    in_=st[:])
                            else:
                                nc.vector.tensor_scalar_add(
                                    out=kT_sb[:, cg, tsl],
                                    in0=st_box["ps"][idx][:],
                                    scalar1=b_sb[:, cg:cg + 1])

                def v_step(pair, ko, st_box):
                    if ko == 0:
                        st_box["ps"] = [
                            ps1.tile([128, 512], F32, tag="ph1",
                                     name=f"pv{tt}")
                            for tt in pair]
                    for idx, tt in enumerate(pair):
                        nc.tensor.matmul(
                            st_box["ps"][idx][:],
                            box["xc"][:, ko, 128 * tt:128 * (tt + 1)],
                            WV_sb[:, ko, :],
                            start=(ko == 0), stop=(ko == KO - 1))
                    if ko == KO - 1:
                        for idx, tt in enumerate(pair):
                            kj = 4 * tci + tt
                            nc.vector.tensor_add(
                                v_aug[:, kj, :, 0:HD],
                                st_box["ps"][idx][:].rearrange(
                                    "p (h d) -> p h d", d=HD),
                                bv_sb[:, :].rearrange(
                                    "p (h d) -> p h d", d=HD))

                for which in ("q", "k"):
                    for pair in ((0, 1), (2, 3)):
                        sb = {}
                        for ko in range(KO):
                            steps.append(functools.partial(
                                qk_step, which, pair, ko, sb))
                for pair in ((0, 1), (2, 3)):
                    sb = {}
                    for ko in range(KO):
                        steps.append(functools.partial(v_step, pair, ko, sb))
                return steps

            # ---- proj step closures for one t-chunk ----
            def proj_steps(jj):
                steps = []

                def do(tt, n, st_box):
                    if n == 0:
                        st_box["ost"] = ostp.tile([128, D], F32, tag="ost", name="ost")
                    po = ps1.tile([128, 512], F32, tag="ph1", name="po")
                    for ko in range(CG):
                        nc.tensor.matmul(
                            po[:],
                            yT_sb[:, ko, 128 * tt:128 * (tt + 1)],
                            WP_sb[:, ko, 512 * n:512 * (n + 1)],
                            start=(ko == 0), stop=(ko == CG - 1))
                    nc.vector.tensor_copy(
                        out=st_box["ost"][:, 512 * n:512 * (n + 1)],
                        in_=po[:])
                    if n == 1:
                        nc.sync.dma_start(out=OUT[128 * tt:128 * (tt + 1), :],
                                          in_=st_box["ost"][:])

                for tt in range(4 * jj, 4 * jj + 4):
                    sb = {}
                    for n in range(2):
                        steps.append(functools.partial(do, tt, n, sb))
                return steps

            # ---- phase-1 for chunk 0 runs up front: interleave the
            # W and x DMAs per-ko so the first matmul starts after ~384KB
            tsl0 = slice(0, 512)
            xc0 = xs.tile([128, KO, 512], BF16, tag="xc", name="xc")
            for ko in range(KO):
                nc.gpsimd.dma_start(out=xc0[:, ko, :],
                                    in_=xbf_r[:, ko, tsl0])
                nc.sync.dma_start(out=WQ_sb[:, ko, :], in_=wq_r[:, ko, :])
            nc.sync.dma_start(out=WK_sb[:], in_=wk_r)
            nc.sync.dma_start(out=bq_sb[:], in_=bq_r)
            nc.sync.dma_start(out=bk_sb[:], in_=bk_r)
            nc.sync.dma_start(out=bv_sb[:], in_=bv_bcast)
            nc.sync.dma_start(out=WV_sb[:], in_=wv_r)
            nc.sync.dma_start(out=masks_sb[:], in_=MASKS[:, :, :])
            nc.sync.dma_start(out=WP_sb[:], in_=wp_r)
            for s in ph1_steps(0, box={"xc": xc0}):
                s()

            # ---- fused attention + fillers ----
            filler_plan = {
                0: lambda: ph1_steps(1),
                1: lambda: ph1_steps(2) + proj_steps(0),
                2: lambda: ph1_steps(3),
                3: lambda: proj_steps(1) + proj_steps(2),
            }
            fillers = []
            for j in range(NCH):
                fillers.extend(filler_plan[j]())
                npairs = 2 * j + 2
                nkj = 4 * j + 4
                total_av_steps = CG * npairs
                nfill = len(fillers)
                done_av = 0
                popped = 0
                for i in range(CG):
                    if j == 0 and i == 0:
                        qc_next = qcp.tile([128, 512], BF16, tag="qc",
                                           name="qc")
                        nc.gpsimd.dma_start(
                            out=qc_next[:], in_=qt_scr[0:128, 0:512])
                    qc = qc_next
                    # prefetch the next head-pair's q tile
                    ni, nj = (i + 1, j) if i + 1 < CG else (0, j + 1)
                    if nj < NCH:
                        qc_next = qcp.tile([128, 512], BF16, tag="qc",
                                           name="qc")
                        nc.gpsimd.dma_start(
                            out=qc_next[:],
                            in_=qt_scr[128 * ni:128 * (ni + 1),
                                       512 * nj:512 * (nj + 1)])
                    oaug = [ps_oa.tile([128, 512], F32, tag="oaug",
                                       name=f"oaug{hh}")
                            for hh in range(2)]
                    pexp = {}

                    def emit_scores(u, i=i, j=j, qc=qc, pexp=pexp):
                        for hh in range(2):
                            base = 64 * hh
                            pps = ps_p.tile([128, 1024], F32, tag="pps",
                                            name=f"pps{hh}")
                            for half in range(2):
                                m = 2 * u + half
                                nc.tensor.matmul(
                                    pps[:, 512 * half:512 * (half + 1)],
                                    kT_sb[base:base + 64, i,
                                          128 * m:128 * (m + 1)],
                                    qc[base:base + 64, :],
                                    start=True, stop=True)
                            pe = pep.tile([128, 1024], BF16, tag="pe",
                                          name=f"pe{hh}")
                            nc.scalar.activation(
                                out=pe[:], in_=pps[:], func=AF.Exp,
                                scale=float(SCALE))
                            if 2 * u >= 4 * j:
                                r0 = 2 * u - 4 * j
                                nc.vector.tensor_mul(
                                    pe[:].rearrange("p (r c) -> p r c",
                                                    c=512),
                                    pe[:].rearrange("p (r c) -> p r c",
                                                    c=512),
                                    masks_sb[:, r0:r0 + 2, :])
                            pexp[(hh, u)] = pe

                    emit_scores(0)
                    for u in range(npairs):
                        if u + 1 < npairs:
                            emit_scores(u + 1)
                        for hh in range(2):
                            h = 2 * i + hh
                            pe = pexp.pop((hh, u))
                            for half in range(2):
                                m = 2 * u + half
                                nc.tensor.matmul(
                                    oaug[hh][:], v_aug[:, m, h, :],
                                    pe[:, 512 * half:512 * (half + 1)],
                                    start=(m == 0), stop=(m == nkj - 1))
                        done_av += 1
                        target = (nfill * done_av) // total_av_steps
                        while popped < target:
                            fillers[popped]()
                            popped += 1
                    # normalization, entirely off the PE/ACT engines;
                    # oaug is released by the first copy so the next
                    # head-pair's AV can start immediately
                    for hh in range(2):
                        base = 64 * hh
                        oc = dnp.tile([HD + 1, 512], F32, tag="oc")
                        nc.vector.tensor_copy(out=oc[:],
                                              in_=oaug[hh][0:HD + 1, :])
                        dnd = dramd.tile([1, 512], F32, tag="dnd")
                        nc.gpsimd.dma_start(out=dnd[:, :],
                                            in_=oc[HD:HD + 1, :])
                        rb = rbp.tile([64, 512], F32, tag="rb")
                        dnd_ap = dnd[:, :]
                        nc.gpsimd.dma_start(
                            out=rb[:],
                            in_=bass.AP(tensor=dnd_ap.tensor,
                                        offset=dnd_ap.offset,
                                        ap=[[0, 64], [1, 512]]))
                        rc = rcp.tile([64, 512], F32, tag="rc")
                        nc.vector.reciprocal_approx_fast(out=rc[:],
                                                         in_=rb[:])
                        nc.vector.tensor_mul(
                            yT_sb[base:base + 64, i,
                                  512 * j:512 * (j + 1)],
                            oc[0:HD, :], rc[:])
                del fillers[:popped]

            for s in proj_steps(NCH - 1):
                s()

    nc.compile()
    return nc


def _masks():
    m = np.zeros((128, 4, 512), dtype=np.float32)
    cols = np.arange(512)
    for r in range(4):
        for p in range(128):
            m[p, r, :] = cols >= (128 * r + p)
    return m


def _prep_inputs(x, Wqkv, bqkv, Wproj, bproj):
    import ml_dtypes
    bf16 = ml_dtypes.bfloat16
    x = np.asarray(x, dtype=np.float32)
    Wqkv = np.asarray(Wqkv, dtype=np.float32)
    bqkv = np.asarray(bqkv, dtype=np.float32)
    Wproj = np.asarray(Wproj, dtype=np.float32)
    masks = _masks()
    in_maps = []
    for c in range(NCORES):
        b, g = c // 2, c % 2
        sl = slice(C * g, C * (g + 1))
        xT = np.ascontiguousarray(x[b].T)
        in_maps.append({
            "XBF": xT.astype(bf16),
            "WQ": np.ascontiguousarray(Wqkv[:, sl]).astype(bf16),
            "WK": np.ascontiguousarray(
                Wqkv[:, D + C * g:D + C * (g + 1)]).astype(bf16),
            "WV": np.ascontiguousarray(
                Wqkv[:, 2 * D + C * g:2 * D + C * (g + 1)]).astype(bf16),
            "BQ": np.ascontiguousarray(bqkv[sl]),
            "BK": np.ascontiguousarray(bqkv[D + C * g:D + C * (g + 1)]),
            "BV": np.ascontiguousarray(bqkv[2 * D + C * g:2 * D + C * (g + 1)]),
            "WP": np.ascontiguousarray(Wproj[sl, :]).astype(bf16),
            "MASKS": masks.astype(bf16),
        })
    return in_maps


def _run(inputs, **run_kwargs):
    from concourse.bass_utils import run_bass_kernel_spmd
    if "nc" not in _CACHE:
        _CACHE["nc"] = _build()
    nc = _CACHE["nc"]
    in_maps = _prep_inputs(**inputs)
    res = run_bass_kernel_spmd(nc, in_maps, core_ids=list(range(NCORES)),
                               **run_kwargs)
    bproj = np.asarray(inputs["bproj"], dtype=np.float32)
    out = np.empty((B, T, D), dtype=np.float32)
    for b in range(B):
        out[b] = res.results[2 * b]["OUT"] + res.results[2 * b + 1]["OUT"]
        out[b] += bproj
    return out, res


def kernel(x, Wqkv, bqkv, Wproj, bproj):
    out, _ = _run(dict(x=x, Wqkv=Wqkv, bqkv=bqkv, Wproj=Wproj, bproj=bproj))
    return out



# revision 2
# speedup vs baseline: 1.0786x; 1.0786x over previous
"""Causal self-attention block on 8 Trainium2 NeuronCores.

Reference computation (B=4, T=2048, D=1024, H=16, hd=64):
    qkv = x @ Wqkv + bqkv ; per-head causal softmax(q k^T / sqrt(hd)) v ;
    out = concat_heads @ Wproj + bproj

Sharding: core c = (batch b = c//2, head-group g = c%2 of 8 heads).
Each core computes its batch's qkv for its 8 heads, the causal attention,
and a partial projection (its 512 rows of Wproj). Host sums the two
head-group partials per batch and adds bproj.

All matmuls run bf16 x bf16 with fp32 PSUM accumulation.

The three phases are fused into one software pipeline per 512-wide t-chunk
so the PE never idles while the scalar engine works through exp().
q^T stays resident in SBUF (no DRAM roundtrip); score/AV matmuls and the
causal mask are trimmed to the live (non-masked) 128-column blocks; the
startup DMAs are ordered so the first matmul issues as soon as WQ[ko0:4]
and x[ko0:2] land.
"""

import numpy as np

B, T, D, H, HD = 4, 2048, 1024, 16, 64
NCORES = 8
HPG = H // 2          # heads per group: 8
C = HPG * HD          # per-core q/k/v columns: 512
KO = D // 128         # 8 input-dim k-tiles
NCH = T // 512        # 4 t-chunks
NT = T // 128         # 16 t-tiles
CG = C // 128         # 4 col-groups per q/k
VW = HD + 1           # AV stationary width: 64 hd cols + 1 ones col
SCALE = 1.0 / np.sqrt(HD)

_CACHE = {}


def _build():
    import functools
    import concourse.mybir as mybir
    import concourse.tile as tile
    from concourse import bacc
    import concourse.bass as bass

    F32 = mybir.dt.float32
    BF16 = mybir.dt.bfloat16
    AF = mybir.ActivationFunctionType

    nc = bacc.Bacc("TRN2", target_bir_lowering=False, debug=False,
                   num_devices=NCORES)
    XBF = nc.declare_dram_parameter("XBF", [D, T], BF16, isOutput=False)
    WQ = nc.declare_dram_parameter("WQ", [D, C], BF16, isOutput=False)
    WK = nc.declare_dram_parameter("WK", [D, C], BF16, isOutput=False)
    WV = nc.declare_dram_parameter("WV", [D, C], BF16, isOutput=False)
    BQ = nc.declare_dram_parameter("BQ", [C], F32, isOutput=False)
    BK = nc.declare_dram_parameter("BK", [C], F32, isOutput=False)
    BV = nc.declare_dram_parameter("BV", [C], F32, isOutput=False)
    WP = nc.declare_dram_parameter("WP", [C, D], BF16, isOutput=False)
    MASKS = nc.declare_dram_parameter("MASKS", [128, 128], BF16,
                                      isOutput=False)
    OUT = nc.declare_dram_parameter("OUT", [T, D], BF16, isOutput=True)

    xbf_r = XBF[:, :].rearrange("(ko p) t -> p ko t", p=128)
    wq_r = WQ[:, :].rearrange("(ko p) c -> p ko c", p=128)
    wk_r = WK[:, :].rearrange("(ko p) c -> p ko c", p=128)
    wv_r = WV[:, :].rearrange("(ko p) c -> p ko c", p=128)
    wp_r = WP[:, :].rearrange("(ko p) c -> p ko c", p=128)
    bq_r = BQ[:].rearrange("(cg p) -> p cg", p=128)
    bk_r = BK[:].rearrange("(cg p) -> p cg", p=128)
    bv_ap = BV[:]
    bv_bcast = bass.AP(tensor=bv_ap.tensor, offset=bv_ap.offset,
                       ap=[[0, 128]] + list(bv_ap.ap))

    from contextlib import ExitStack
    with tile.TileContext(nc) as tc:
        with ExitStack() as ctx:
            def pool(name, bufs, space="SBUF"):
                return ctx.enter_context(
                    tc.tile_pool(name=name, bufs=bufs, space=space))
            consts = pool("consts", 1)
            ktp = pool("ktp", 1)
            qtp = pool("qtp", 1)
            vtp = pool("vtp", 1)
            ytp = pool("ytp", 1)
            wp1 = pool("wp1", 1)
            xs = pool("xs", 2)
            pep = pool("pe", 4)
            rbp = pool("rb", 2)
            rcp = pool("rc", 2)
            ostp = pool("ost", 2)
            dnp = pool("dn", 3)
            dramd = pool("dramd", 4, space="DRAM")
            ps1 = pool("ps1", 2, space="PSUM")
            ps_p = pool("psp", 2, space="PSUM")
            ps_oa = pool("psa", 2, space="PSUM")

            WQ_sb = wp1.tile([128, KO, C], BF16)
            WK_sb = wp1.tile([128, KO, C], BF16)
            WV_sb = wp1.tile([128, KO, C], BF16)
            WP_sb = wp1.tile([128, CG, D], BF16)

            masks_sb = consts.tile([128, 128], BF16)
            bq_sb = consts.tile([128, CG], F32)
            bk_sb = consts.tile([128, CG], F32)
            bv_sb = consts.tile([128, C], F32)

            kT_sb = ktp.tile([128, CG, T], BF16)      # [kcol%128, cg, t]
            qT_sb = qtp.tile([128, CG, T], BF16)      # [qcol%128, cg, t]
            v_aug = vtp.tile([128, NT, HPG, VW], BF16)
            yT_sb = ytp.tile([128, CG, T], BF16)      # [ycol%128, cg, t]

            # ---- phase-1 step closures for one t-chunk (run as fillers) ---
            def ph1_steps(tci, box=None):
                tsl = slice(512 * tci, 512 * (tci + 1))
                if box is None:
                    box = {}

                def load_x():
                    if "xc" in box:
                        return
                    box["xc"] = xs.tile([128, KO, 512], BF16, tag="xc",
                                        name="xc")
                    nc.gpsimd.dma_start(out=box["xc"][:],
                                        in_=xbf_r[:, :, tsl])

                steps = [load_x]

                def qk_step(which, pair, ko, st_box):
                    W_sb = WQ_sb if which == "q" else WK_sb
                    b_sb = bq_sb if which == "q" else bk_sb
                    o_sb = qT_sb if which == "q" else kT_sb
                    if ko == 0:
                        st_box["ps"] = [
                            ps1.tile([128, 512], F32, tag="ph1",
                                     name=f"p{which}{cg}")
                            for cg in pair]
                    for idx, cg in enumerate(pair):
                        nc.tensor.matmul(
                            st_box["ps"][idx][:],
                            W_sb[:, ko, 128 * cg:128 * (cg + 1)],
                            box["xc"][:, ko, :],
                            start=(ko == 0), stop=(ko == KO - 1))
                    if ko == KO - 1:
                        for idx, cg in enumerate(pair):
                            nc.vector.tensor_scalar_add(
                                out=o_sb[:, cg, tsl],
                                in0=st_box["ps"][idx][:],
                                scalar1=b_sb[:, cg:cg + 1])

                def v_step(pair, ko, st_box):
                    if ko == 0:
                        st_box["ps"] = [
                            ps1.tile([128, 512], F32, tag="ph1",
                                     name=f"pv{tt}")
                            for tt in pair]
                    for idx, tt in enumerate(pair):
                        nc.tensor.matmul(
                            st_box["ps"][idx][:],
                            box["xc"][:, ko, 128 * tt:128 * (tt + 1)],
                            WV_sb[:, ko, :],
                            start=(ko == 0), stop=(ko == KO - 1))
                    if ko == KO - 1:
                        for idx, tt in enumerate(pair):
                            kj = 4 * tci + tt
                            nc.vector.tensor_add(
                                v_aug[:, kj, :, 0:HD],
                                st_box["ps"][idx][:].rearrange(
                                    "p (h d) -> p h d", d=HD),
                                bv_sb[:, :].rearrange(
                                    "p (h d) -> p h d", d=HD))

                for which in ("q", "k"):
                    for pair in ((0, 1), (2, 3)):
                        sb = {}
                        for ko in range(KO):
                            steps.append(functools.partial(
                                qk_step, which, pair, ko, sb))
                for pair in ((0, 1), (2, 3)):
                    sb = {}
                    for ko in range(KO):
                        steps.append(functools.partial(v_step, pair, ko, sb))
                return steps

            # ---- proj step closures for one t-chunk ----
            def proj_steps(jj):
                steps = []

                def do(tt, n, st_box):
                    if n == 0:
                        st_box["ost"] = ostp.tile([128, D], BF16, tag="ost",
                                                  name="ost")
                    po = ps1.tile([128, 512], F32, tag="ph1", name="po")
                    for ko in range(CG):
                        nc.tensor.matmul(
                            po[:],
                            yT_sb[:, ko, 128 * tt:128 * (tt + 1)],
                            WP_sb[:, ko, 512 * n:512 * (n + 1)],
                            start=(ko == 0), stop=(ko == CG - 1))
                    nc.vector.tensor_copy(
                        out=st_box["ost"][:, 512 * n:512 * (n + 1)],
                        in_=po[:])
                    if n == 1:
                        nc.sync.dma_start(out=OUT[128 * tt:128 * (tt + 1), :],
                                          in_=st_box["ost"][:])

                for tt in range(4 * jj, 4 * jj + 4):
                    sb = {}
                    for n in range(2):
                        steps.append(functools.partial(do, tt, n, sb))
                return steps

            # ---- startup: priority-ordered DMAs so the first matmul can
            # issue after ~640KB instead of after the whole weight set ----
            xc0 = xs.tile([128, KO, 512], BF16, tag="xc", name="xc")
            nc.gpsimd.dma_start(out=xc0[:, 0:2, :], in_=xbf_r[:, 0:2, 0:512])
            nc.sync.dma_start(out=WQ_sb[:, 0:4, :], in_=wq_r[:, 0:4, :])
            nc.gpsimd.dma_start(out=xc0[:, 2:8, :], in_=xbf_r[:, 2:8, 0:512])
            nc.scalar.dma_start(out=bq_sb[:], in_=bq_r)
            nc.scalar.dma_start(out=bk_sb[:], in_=bk_r)
            nc.scalar.dma_start(out=bv_sb[:], in_=bv_bcast)
            nc.scalar.dma_start(out=masks_sb[:], in_=MASKS[:, :])
            nc.sync.dma_start(out=WQ_sb[:, 4:8, :], in_=wq_r[:, 4:8, :])
            nc.sync.dma_start(out=WK_sb[:], in_=wk_r)
            nc.sync.dma_start(out=WV_sb[:], in_=wv_r)
            nc.sync.dma_start(out=WP_sb[:], in_=wp_r)
            nc.vector.memset(v_aug[:, :, :, HD:VW], 1.0)
            for s in ph1_steps(0, box={"xc": xc0}):
                s()

            # ---- fused attention + fillers ----
            filler_plan = {
                0: lambda: ph1_steps(1),
                1: lambda: ph1_steps(2),
                2: lambda: ph1_steps(3),
                3: lambda: proj_steps(0) + proj_steps(1) + proj_steps(2),
            }
            fillers = []
            for j in range(NCH):
                fillers.extend(filler_plan[j]())
                npairs = 2 * j + 2
                nkj = 4 * j + 4
                total_av_steps = CG * npairs
                nfill = len(fillers)
                done_av = 0
                popped = 0
                for i in range(CG):
                    qc = qT_sb[:, i, 512 * j:512 * (j + 1)]
                    oaug = [ps_oa.tile([128, 512], F32, tag="oaug",
                                       name=f"oaug{hh}")
                            for hh in range(2)]
                    pexp = {}

                    def emit_scores(u, i=i, j=j, qc=qc, pexp=pexp):
                        for hh in range(2):
                            base = 64 * hh
                            pps = ps_p.tile([128, 1024], F32, tag="pps",
                                            name=f"pps{hh}")
                            for half in range(2):
                                m = 2 * u + half
                                q0 = 128 * (m - 4 * j) if m >= 4 * j else 0
                                nc.tensor.matmul(
                                    pps[:, 512 * half + q0:512 * (half + 1)],
                                    kT_sb[base:base + 64, i,
                                          128 * m:128 * (m + 1)],
                                    qc[base:base + 64, q0:512],
                                    start=True, stop=True)
                            pe = pep.tile([128, 1024], BF16, tag="pe",
                                          name=f"pe{hh}")
                            nc.scalar.activation(
                                out=pe[:], in_=pps[:], func=AF.Exp,
                                scale=float(SCALE))
                            if 2 * u >= 4 * j:
                                for half in range(2):
                                    m = 2 * u + half
                                    col = 512 * half + 128 * (m - 4 * j)
                                    nc.vector.tensor_mul(
                                        pe[:, col:col + 128],
                                        pe[:, col:col + 128],
                                        masks_sb[:, :])
                            pexp[(hh, u)] = pe

                    emit_scores(0)
                    for u in range(npairs):
                        if u + 1 < npairs:
                            emit_scores(u + 1)
                        for hh in range(2):
                            h = 2 * i + hh
                            pe = pexp.pop((hh, u))
                            for half in range(2):
                                m = 2 * u + half
                                q0 = 128 * (m - 4 * j) if m >= 4 * j else 0
                                nc.tensor.matmul(
                                    oaug[hh][0:VW, q0:512],
                                    v_aug[:, m, h, :],
                                    pe[:, 512 * half + q0:512 * (half + 1)],
                                    start=(m == 0), stop=(m == nkj - 1),
                                    skip_group_check=True)
                        done_av += 1
                        target = (nfill * done_av) // total_av_steps
                        while popped < target:
                            fillers[popped]()
                            popped += 1
                    # normalization, entirely off the PE/ACT engines;
                    # oaug is released by the two copies so the next
                    # head-pair's AV can start immediately
                    for hh in range(2):
                        base = 64 * hh
                        dnf = dnp.tile([1, 512], F32, tag="dnf", name="dnf")
                        nc.vector.tensor_copy(out=dnf[:],
                                              in_=oaug[hh][HD:HD + 1, :])
                        oc = dnp.tile([HD, 512], BF16, tag="oc", name="oc")
                        nc.vector.tensor_copy(out=oc[:],
                                              in_=oaug[hh][0:HD, :])
                        dnd = dramd.tile([1, 512], F32, tag="dnd")
                        nc.gpsimd.dma_start(out=dnd[:, :], in_=dnf[:, :])
                        rb = rbp.tile([64, 512], F32, tag="rb")
                        dnd_ap = dnd[:, :]
                        nc.gpsimd.dma_start(
                            out=rb[:],
                            in_=bass.AP(tensor=dnd_ap.tensor,
                                        offset=dnd_ap.offset,
                                        ap=[[0, 64], [1, 512]]))
                        rc = rcp.tile([64, 512], F32, tag="rc")
                        nc.vector.reciprocal_approx_fast(out=rc[:],
                                                         in_=rb[:])
                        nc.vector.tensor_mul(
                            yT_sb[base:base + 64, i,
                                  512 * j:512 * (j + 1)],
                            oc[:], rc[:])
                del fillers[:popped]

            for s in proj_steps(NCH - 1):
                s()

    nc.compile()
    return nc


def _masks():
    m = np.zeros((128, 128), dtype=np.float32)
    cols = np.arange(128)
    for p in range(128):
        m[p, :] = cols >= p
    return m


def _prep_inputs(x, Wqkv, bqkv, Wproj, bproj):
    import ml_dtypes
    bf16 = ml_dtypes.bfloat16
    x = np.asarray(x, dtype=np.float32)
    Wqkv = np.asarray(Wqkv, dtype=np.float32)
    bqkv = np.asarray(bqkv, dtype=np.float32)
    Wproj = np.asarray(Wproj, dtype=np.float32)
    masks = _masks()
    in_maps = []
    for c in range(NCORES):
        b, g = c // 2, c % 2
        sl = slice(C * g, C * (g + 1))
        xT = np.ascontiguousarray(x[b].T)
        in_maps.append({
            "XBF": xT.astype(bf16),
            "WQ": np.ascontiguousarray(Wqkv[:, sl]).astype(bf16),
            "WK": np.ascontiguousarray(
                Wqkv[:, D + C * g:D + C * (g + 1)]).astype(bf16),
            "WV": np.ascontiguousarray(
                Wqkv[:, 2 * D + C * g:2 * D + C * (g + 1)]).astype(bf16),
            "BQ": np.ascontiguousarray(bqkv[sl]),
            "BK": np.ascontiguousarray(bqkv[D + C * g:D + C * (g + 1)]),
            "BV": np.ascontiguousarray(bqkv[2 * D + C * g:2 * D + C * (g + 1)]),
            "WP": np.ascontiguousarray(Wproj[sl, :]).astype(bf16),
            "MASKS": masks.astype(bf16),
        })
    return in_maps


def _run(inputs, **run_kwargs):
    from concourse.bass_utils import run_bass_kernel_spmd
    if "nc" not in _CACHE:
        _CACHE["nc"] = _build()
    nc = _CACHE["nc"]
    in_maps = _prep_inputs(**inputs)
    res = run_bass_kernel_spmd(nc, in_maps, core_ids=list(range(NCORES)),
                               **run_kwargs)
    bproj = np.asarray(inputs["bproj"], dtype=np.float32)
    out = np.empty((B, T, D), dtype=np.float32)
    for b in range(B):
        out[b] = (res.results[2 * b]["OUT"].astype(np.float32)
                  + res.results[2 * b + 1]["OUT"].astype(np.float32))
        out[b] += bproj
    return out, res


def kernel(x, Wqkv, bqkv, Wproj, bproj):
    out, _ = _run(dict(x=x, Wqkv=Wqkv, bqkv=bqkv, Wproj=Wproj, bproj=bproj))
    return out


# revision 27
# speedup vs baseline: 1.1106x; 1.0297x over previous
"""Causal self-attention block on 8 Trainium2 NeuronCores.

Reference computation (B=4, T=2048, D=1024, H=16, hd=64):
    qkv = x @ Wqkv + bqkv ; per-head causal softmax(q k^T / sqrt(hd)) v ;
    out = concat_heads @ Wproj + bproj

Sharding: core c = (batch b = c//2, head-group g = c%2 of 8 heads).
Each core computes its batch's qkv for its 8 heads, the causal attention,
and a partial projection (its 512 rows of Wproj). Host sums the two
head-group partials per batch and adds bproj.

All matmuls run bf16 x bf16 with fp32 PSUM accumulation.

The three phases are fused into one software pipeline per 512-wide t-chunk
so the PE never idles while the scalar engine works through exp().
q^T stays resident in SBUF (no DRAM roundtrip); score/AV matmuls and the
causal mask are trimmed to the live (non-masked) 128-column blocks; the
startup DMAs are ordered so the first matmul issues as soon as WQ[ko0:4]
and x[ko0:2] land.
"""

import numpy as np

B, T, D, H, HD = 4, 2048, 1024, 16, 64
NCORES = 8
HPG = H // 2          # heads per group: 8
C = HPG * HD          # per-core q/k/v columns: 512
KO = D // 128         # 8 input-dim k-tiles
NCH = T // 512        # 4 t-chunks
NT = T // 128         # 16 t-tiles
CG = C // 128         # 4 col-groups per q/k
VW = HD + 1           # AV stationary width: 64 hd cols + 1 ones col
SCALE = 1.0 / np.sqrt(HD)

_CACHE = {}


def _build():
    import functools
    import concourse.mybir as mybir
    import concourse.tile as tile
    from concourse import bacc
    import concourse.bass as bass

    F32 = mybir.dt.float32
    BF16 = mybir.dt.bfloat16
    AF = mybir.ActivationFunctionType

    nc = bacc.Bacc("TRN2", target_bir_lowering=False, debug=False,
                   num_devices=NCORES)
    # all tensors are pre-arranged on the host to "one contiguous line per
    # SBUF partition" so every DMA is <=128 descriptor lines (issue time on
    # the queue engines is ~9ns/line and dominates startup otherwise)
    XBF = nc.declare_dram_parameter("XBF", [128, NCH, KO, 512], BF16,
                                    isOutput=False)
    WQ = nc.declare_dram_parameter("WQ", [128, KO, C], BF16, isOutput=False)
    WK = nc.declare_dram_parameter("WK", [128, KO, C], BF16, isOutput=False)
    WV = nc.declare_dram_parameter("WV", [128, KO, C], BF16, isOutput=False)
    BQ = nc.declare_dram_parameter("BQ", [C], F32, isOutput=False)
    BK = nc.declare_dram_parameter("BK", [C], F32, isOutput=False)
    BV = nc.declare_dram_parameter("BV", [C], F32, isOutput=False)
    WP = nc.declare_dram_parameter("WP", [128, CG, D], BF16, isOutput=False)
    MASKS = nc.declare_dram_parameter("MASKS", [128, 128], BF16,
                                      isOutput=False)
    OUT = nc.declare_dram_parameter("OUT", [T, D], BF16, isOutput=True)

    bq_r = BQ[:].rearrange("(cg p) -> p cg", p=128)
    bk_r = BK[:].rearrange("(cg p) -> p cg", p=128)
    bv_ap = BV[:]
    bv_bcast = bass.AP(tensor=bv_ap.tensor, offset=bv_ap.offset,
                       ap=[[0, 128]] + list(bv_ap.ap))

    from contextlib import ExitStack
    with tile.TileContext(nc) as tc:
        with ExitStack() as ctx:
            def pool(name, bufs, space="SBUF"):
                return ctx.enter_context(
                    tc.tile_pool(name=name, bufs=bufs, space=space))
            consts = pool("consts", 1)
            ktp = pool("ktp", 1)
            qtp = pool("qtp", 1)
            vtp = pool("vtp", 1)
            ytp = pool("ytp", 1)
            wp1 = pool("wp1", 1)
            xs = pool("xs", 2)
            pep = pool("pe", 4)
            rbp = pool("rb", 2)
            rcp = pool("rc", 2)
            ostp = pool("ost", 2)
            dnp = pool("dn", 3)
            dramd = pool("dramd", 4, space="DRAM")
            ps1 = pool("ps1", 2, space="PSUM")
            ps_p = pool("psp", 2, space="PSUM")
            ps_oa = pool("psa", 2, space="PSUM")

            WQ_sb = wp1.tile([128, KO, C], BF16)
            WK_sb = wp1.tile([128, KO, C], BF16)
            WV_sb = wp1.tile([128, KO, C], BF16)
            WP_sb = wp1.tile([128, CG, D], BF16)

            masks_sb = consts.tile([128, 128], BF16)
            bq_sb = consts.tile([128, CG], F32)
            bk_sb = consts.tile([128, CG], F32)
            bv_sb = consts.tile([128, C], F32)

            kT_sb = ktp.tile([128, CG, T], BF16)      # [kcol%128, cg, t]
            qT_sb = qtp.tile([128, CG, T], BF16)      # [qcol%128, cg, t]
            v_aug = vtp.tile([128, NT, HPG, VW], BF16)
            yT_sb = ytp.tile([128, CG, T], BF16)      # [ycol%128, cg, t]

            # ---- phase-1 step closures for one t-chunk (run as fillers) ---
            def ph1_steps(tci, box=None, parts="all"):
                tsl = slice(512 * tci, 512 * (tci + 1))
                if box is None:
                    box = {}

                def load_x():
                    if "xc" in box:
                        return
                    box["xc"] = xs.tile([128, KO, 512], BF16, tag="xc",
                                        name="xc")
                    nc.gpsimd.dma_start(out=box["xc"][:],
                                        in_=XBF[:, tci, :, :])

                steps = [load_x]

                def qk_step(which, pair, ko, st_box):
                    W_sb = WQ_sb if which == "q" else WK_sb
                    b_sb = bq_sb if which == "q" else bk_sb
                    o_sb = qT_sb if which == "q" else kT_sb
                    if ko == 0:
                        st_box["ps"] = [
                            ps1.tile([128, 512], F32, tag="ph1",
                                     name=f"p{which}{cg}")
                            for cg in pair]
                    for idx, cg in enumerate(pair):
                        nc.tensor.matmul(
                            st_box["ps"][idx][:],
                            W_sb[:, ko, 128 * cg:128 * (cg + 1)],
                            box["xc"][:, ko, :],
                            start=(ko == 0), stop=(ko == KO - 1))
                    if ko == KO - 1:
                        for idx, cg in enumerate(pair):
                            nc.vector.tensor_scalar_add(
                                out=o_sb[:, cg, tsl],
                                in0=st_box["ps"][idx][:],
                                scalar1=b_sb[:, cg:cg + 1])

                def v_step(pair, ko, st_box):
                    if ko == 0:
                        st_box["ps"] = [
                            ps1.tile([128, 512], F32, tag="ph1",
                                     name=f"pv{tt}")
                            for tt in pair]
                    for idx, tt in enumerate(pair):
                        nc.tensor.matmul(
                            st_box["ps"][idx][:],
                            box["xc"][:, ko, 128 * tt:128 * (tt + 1)],
                            WV_sb[:, ko, :],
                            start=(ko == 0), stop=(ko == KO - 1))
                    if ko == KO - 1:
                        for idx, tt in enumerate(pair):
                            kj = 4 * tci + tt
                            nc.vector.tensor_add(
                                v_aug[:, kj, :, 0:HD],
                                st_box["ps"][idx][:].rearrange(
                                    "p (h d) -> p h d", d=HD),
                                bv_sb[:, :].rearrange(
                                    "p (h d) -> p h d", d=HD))

                def add_qk(which, pairs):
                    for pair in pairs:
                        sb = {}
                        for ko in range(KO):
                            steps.append(functools.partial(
                                qk_step, which, pair, ko, sb))

                def add_v():
                    for pair in ((0, 1), (2, 3)):
                        sb = {}
                        for ko in range(KO):
                            steps.append(functools.partial(
                                v_step, pair, ko, sb))

                if parts == "all":
                    add_qk("q", ((0, 1), (2, 3)))
                    add_qk("k", ((0, 1), (2, 3)))
                    add_v()
                elif parts == "lead0":       # minimal prefix for chunk 0
                    add_qk("q", ((0, 1),))
                    add_qk("k", ((0, 1),))
                    add_v()
                elif parts == "tail0":       # rest of chunk 0, as fillers
                    steps.pop(0)             # x already loaded
                    add_qk("q", ((2, 3),))
                    add_qk("k", ((2, 3),))
                return steps

            # ---- proj step closures for one t-chunk ----
            def proj_steps(jj):
                steps = []

                def do(tt, n, st_box):
                    if n == 0:
                        st_box["ost"] = ostp.tile([128, D], BF16, tag="ost",
                                                  name="ost")
                    po = ps1.tile([128, 512], F32, tag="ph1", name="po")
                    for ko in range(CG):
                        nc.tensor.matmul(
                            po[:],
                            yT_sb[:, ko, 128 * tt:128 * (tt + 1)],
                            WP_sb[:, ko, 512 * n:512 * (n + 1)],
                            start=(ko == 0), stop=(ko == CG - 1))
                    nc.vector.tensor_copy(
                        out=st_box["ost"][:, 512 * n:512 * (n + 1)],
                        in_=po[:])
                    if n == 1:
                        nc.sync.dma_start(out=OUT[128 * tt:128 * (tt + 1), :],
                                          in_=st_box["ost"][:])

                for tt in range(4 * jj, 4 * jj + 4):
                    sb = {}
                    for n in range(2):
                        steps.append(functools.partial(do, tt, n, sb))
                return steps

            # ---- last-chunk proj split: partial column-group sums land in
            # SBUF while later head-pairs' attention still runs; the
            # post-loop tail is then only one cg3 matmul + fused add per
            # (tt, n) half ----
            proj3_parts = {}

            def proj3_partial_steps():
                steps = []

                def part(tt, n):
                    po = ps1.tile([128, 512], F32, tag="ph1", name="po")
                    for ko in range(CG - 1):
                        nc.tensor.matmul(
                            po[:],
                            yT_sb[:, ko, 128 * tt:128 * (tt + 1)],
                            WP_sb[:, ko, 512 * n:512 * (n + 1)],
                            start=(ko == 0), stop=(ko == CG - 2))
                    pp = ostp.tile([128, 512], F32, tag="pp", name="pp",
                                   bufs=8)
                    nc.vector.tensor_copy(out=pp[:], in_=po[:])
                    proj3_parts[(tt, n)] = pp

                for tt in range(12, 16):
                    for n in range(2):
                        steps.append(functools.partial(part, tt, n))
                return steps

            def proj3_tail():
                for tt in range(12, 16):
                    ost = ostp.tile([128, D], BF16, tag="ost", name="ost")
                    for n in range(2):
                        po = ps1.tile([128, 512], F32, tag="ph1", name="po")
                        nc.tensor.matmul(
                            po[:],
                            yT_sb[:, CG - 1, 128 * tt:128 * (tt + 1)],
                            WP_sb[:, CG - 1, 512 * n:512 * (n + 1)],
                            start=True, stop=True)
                        nc.vector.scalar_tensor_tensor(
                            out=ost[:, 512 * n:512 * (n + 1)],
                            in0=po[:], scalar=0.0,
                            in1=proj3_parts[(tt, n)][:],
                            op0=mybir.AluOpType.add,
                            op1=mybir.AluOpType.add)
                    nc.sync.dma_start(out=OUT[128 * tt:128 * (tt + 1), :],
                                      in_=ost[:])

            # ---- warm the PE clock (HAM) with a dummy matmul stream while
            # the first weight/x DMAs are still in flight ----
            warm_sb = consts.tile([128, 512], BF16)
            nc.vector.memset(warm_sb[:], 1.0)
            ones_f32 = consts.tile([1, 64], F32)
            nc.vector.memset(ones_f32[:], 1.0)
            wp_ps = ps1.tile([128, 512], F32, tag="ph1", name="warm")
            for _ in range(40):
                nc.tensor.matmul(wp_ps[:], warm_sb[:, 0:128], warm_sb[:],
                                 start=True, stop=True)
            nc.vector.tensor_copy(out=warm_sb[:, 0:1], in_=wp_ps[:, 0:1])

            # ---- startup: priority-ordered DMAs so the first matmul can
            # issue after ~640KB instead of after the whole weight set ----
            xc0 = xs.tile([128, KO, 512], BF16, tag="xc", name="xc")
            nc.gpsimd.dma_start(out=xc0[:, 0:2, :], in_=XBF[:, 0, 0:2, :])
            nc.sync.dma_start(out=WQ_sb[:, 0:2, :], in_=WQ[:, 0:2, :])
            nc.gpsimd.dma_start(out=xc0[:, 2:8, :], in_=XBF[:, 0, 2:8, :])
            nc.sync.dma_start(out=WQ_sb[:, 2:8, :], in_=WQ[:, 2:8, :])
            nc.scalar.dma_start(out=bq_sb[:], in_=bq_r)
            nc.scalar.dma_start(out=bk_sb[:], in_=bk_r)
            nc.scalar.dma_start(out=bv_sb[:], in_=bv_bcast)
            nc.scalar.dma_start(out=masks_sb[:], in_=MASKS[:, :])
            nc.sync.dma_start(out=WK_sb[:, 0:4, :], in_=WK[:, 0:4, :])
            nc.sync.dma_start(out=WK_sb[:, 4:8, :], in_=WK[:, 4:8, :])
            nc.sync.dma_start(out=WV_sb[:], in_=WV[:, :, :])
            nc.sync.dma_start(out=WP_sb[:], in_=WP[:, :, :])
            nc.vector.memset(v_aug[:, :, :, HD:VW], 1.0)
            box0 = {"xc": xc0}
            for s in ph1_steps(0, box=box0, parts="lead0"):
                s()

            # ---- fused attention + fillers ----
            # fillers are (min_done_av, fn): fn must not be popped before
            # done_av reaches the threshold (dependency on a norm result)
            def th(steps, min_done=0):
                return [(min_done, s) for s in steps]

            filler_plan = {
                0: lambda: th(ph1_steps(0, box=box0, parts="tail0")
                              + ph1_steps(1)),
                1: lambda: th(ph1_steps(2)),
                2: lambda: th(ph1_steps(3)),
                3: lambda: (th(proj_steps(0) + proj_steps(1)
                               + proj_steps(2))
                            + th(proj3_partial_steps(), min_done=25)),
            }
            fillers = []
            for j in range(NCH):
                fillers.extend(filler_plan[j]())
                npairs = 2 * j + 2
                nkj = 4 * j + 4
                total_av_steps = CG * npairs
                nfill = len(fillers)
                done_av = 0
                popped = 0
                for i in range(CG):
                    qc = qT_sb[:, i, 512 * j:512 * (j + 1)]
                    oaug = [ps_oa.tile([128, 512], F32, tag="oaug",
                                       name=f"oaug{hh}")
                            for hh in range(2)]
                    pexp = {}

                    def emit_scores(u, i=i, j=j, qc=qc, pexp=pexp):
                        for hh in range(2):
                            base = 64 * hh
                            pps = ps_p.tile([128, 1024], F32, tag="pps",
                                            name=f"pps{hh}")
                            for half in range(2):
                                m = 2 * u + half
                                q0 = 128 * (m - 4 * j) if m >= 4 * j else 0
                                nc.tensor.matmul(
                                    pps[:, 512 * half + q0:512 * (half + 1)],
                                    kT_sb[base:base + 64, i,
                                          128 * m:128 * (m + 1)],
                                    qc[base:base + 64, q0:512],
                                    start=True, stop=True)
                            pe = pep.tile([128, 1024], BF16, tag="pe",
                                          name=f"pe{hh}")
                            # skip exp over leading fully-masked columns
                            # (their scores were never computed)
                            a0 = (128 * (2 * u - 4 * j)
                                  if 2 * u > 4 * j else 0)
                            nc.scalar.activation(
                                out=pe[:, a0:], in_=pps[:, a0:],
                                func=AF.Exp, scale=float(SCALE))
                            if 2 * u >= 4 * j:
                                # one DVE op masks both halves' diagonal
                                # 128-blocks (cols c0 and c0+640)
                                c0 = 128 * (2 * u - 4 * j)
                                blk = pe[:, c0:c0 + 128]
                                pe2 = bass.AP(
                                    tensor=blk.tensor, offset=blk.offset,
                                    ap=[list(blk.ap[0]), [640, 2],
                                        list(blk.ap[-1])])
                                mb = masks_sb[:, :]
                                mk2 = bass.AP(
                                    tensor=mb.tensor, offset=mb.offset,
                                    ap=[list(mb.ap[0]), [0, 2],
                                        list(mb.ap[-1])])
                                nc.vector.tensor_mul(pe2, pe2, mk2)
                            pexp[(hh, u)] = pe

                    emit_scores(0)
                    for u in range(npairs):
                        if u + 1 < npairs:
                            emit_scores(u + 1)
                        for hh in range(2):
                            h = 2 * i + hh
                            pe = pexp.pop((hh, u))
                            for half in range(2):
                                m = 2 * u + half
                                q0 = 128 * (m - 4 * j) if m >= 4 * j else 0
                                nc.tensor.matmul(
                                    oaug[hh][0:VW, q0:512],
                                    v_aug[:, m, h, :],
                                    pe[:, 512 * half + q0:512 * (half + 1)],
                                    start=(m == 0), stop=(m == nkj - 1),
                                    skip_group_check=True)
                        done_av += 1
                        target = (nfill * done_av) // total_av_steps
                        while (popped < target
                               and fillers[popped][0] <= done_av):
                            fillers[popped][1]()
                            popped += 1
                    # normalization, off the PE/ACT engines mid-body (the
                    # denominator row is partition-broadcast via a DRAM
                    # roundtrip); for the very last head-pair the roundtrip
                    # latency (~6us) would gate the final projection, so
                    # broadcast with a K=1 PE matmul instead
                    last = (j == NCH - 1 and i == CG - 1)
                    for hh in range(2):
                        base = 64 * hh
                        dnf = dnp.tile([1, 512], F32, tag="dnf", name="dnf")
                        nc.vector.tensor_copy(out=dnf[:],
                                              in_=oaug[hh][HD:HD + 1, :])
                        oc = dnp.tile([HD, 512], BF16, tag="oc", name="oc")
                        nc.vector.tensor_copy(out=oc[:],
                                              in_=oaug[hh][0:HD, :])
                        if last:
                            bc = ps_oa.tile([128, 512], F32, tag="oaug",
                                            name="bc")
                            nc.tensor.matmul(bc[0:64, :],
                                             ones_f32[0:1, :],
                                             dnf[0:1, :],
                                             start=True, stop=True)
                            rb_ap = bc[0:64, :]
                        else:
                            dnd = dramd.tile([1, 512], F32, tag="dnd")
                            nc.gpsimd.dma_start(out=dnd[:, :], in_=dnf[:, :])
                            rb = rbp.tile([64, 512], F32, tag="rb")
                            dnd_ap = dnd[:, :]
                            nc.gpsimd.dma_start(
                                out=rb[:],
                                in_=bass.AP(tensor=dnd_ap.tensor,
                                            offset=dnd_ap.offset,
                                            ap=[[0, 64], [1, 512]]))
                            rb_ap = rb[:]
                        rc = rcp.tile([64, 512], F32, tag="rc")
                        nc.vector.reciprocal_approx_fast(out=rc[:],
                                                         in_=rb_ap)
                        nc.vector.tensor_mul(
                            yT_sb[base:base + 64, i,
                                  512 * j:512 * (j + 1)],
                            oc[:], rc[:])
                del fillers[:popped]

            proj3_tail()

    nc.compile()
    return nc


def _masks():
    m = np.zeros((128, 128), dtype=np.float32)
    cols = np.arange(128)
    for p in range(128):
        m[p, :] = cols >= p
    return m


def _lines_w(w):
    # [D, C'] -> [128, KO, C']: row p holds W[ko*128+p, :] for all ko
    ko, cc = w.shape[0] // 128, w.shape[1]
    return np.ascontiguousarray(
        w.reshape(ko, 128, cc).transpose(1, 0, 2))


def _prep_inputs(x, Wqkv, bqkv, Wproj, bproj):
    import ml_dtypes
    bf16 = ml_dtypes.bfloat16
    x = np.asarray(x, dtype=np.float32)
    Wqkv = np.asarray(Wqkv, dtype=np.float32)
    bqkv = np.asarray(bqkv, dtype=np.float32)
    Wproj = np.asarray(Wproj, dtype=np.float32)
    masks = _masks()
    in_maps = []
    for c in range(NCORES):
        b, g = c // 2, c % 2
        sl = slice(C * g, C * (g + 1))
        # x^T [D, T] -> [128, NCH, KO, 512]
        xT = x[b].T.reshape(KO, 128, NCH, 512).transpose(1, 2, 0, 3)
        in_maps.append({
            "XBF": np.ascontiguousarray(xT).astype(bf16),
            "WQ": _lines_w(Wqkv[:, sl]).astype(bf16),
            "WK": _lines_w(Wqkv[:, D + C * g:D + C * (g + 1)]).astype(bf16),
            "WV": _lines_w(
                Wqkv[:, 2 * D + C * g:2 * D + C * (g + 1)]).astype(bf16),
            "BQ": np.ascontiguousarray(bqkv[sl]),
            "BK": np.ascontiguousarray(bqkv[D + C * g:D + C * (g + 1)]),
            "BV": np.ascontiguousarray(bqkv[2 * D + C * g:2 * D + C * (g + 1)]),
            "WP": _lines_w(Wproj[sl, :]).astype(bf16),
            "MASKS": masks.astype(bf16),
        })
    return in_maps


def _run(inputs, **run_kwargs):
    from concourse.bass_utils import run_bass_kernel_spmd
    if "nc" not in _CACHE:
        _CACHE["nc"] = _build()
    nc = _CACHE["nc"]
    in_maps = _prep_inputs(**inputs)
    res = run_bass_kernel_spmd(nc, in_maps, core_ids=list(range(NCORES)),
                               **run_kwargs)
    bproj = np.asarray(inputs["bproj"], dtype=np.float32)
    out = np.empty((B, T, D), dtype=np.float32)
    for b in range(B):
        out[b] = (res.results[2 * b]["OUT"].astype(np.float32)
                  + res.results[2 * b + 1]["OUT"].astype(np.float32))
        out[b] += bproj
    return out, res


def kernel(x, Wqkv, bqkv, Wproj, bproj):
    out, _ = _run(dict(x=x, Wqkv=Wqkv, bqkv=bqkv, Wproj=Wproj, bproj=bproj))
    return out
